# revision 1
# baseline (speedup 1.0000x reference)
"""GCN layer (improved self-loops) on 8 Trainium2 NeuronCores.

out = D^{-1/2} (A + 2I) D^{-1/2} X W + b,  deg = in_count + 2.

Strategy (SPMD, one program for all 8 cores; only input data differs per core):
  - Nodes sharded by destination: core m owns rows [m*12544, (m+1)*12544).
  - Aggregation BEFORE the matmul: agg[j] = sum_{e->j} n2_e x[src_e] (+ self),
    out = agg @ W + b, with n2_e = dinv[src] dinv[dst] (2 dinv^2 for self).
  - Per-edge gather of x rows via the custom SWDGE dma_gather instruction
    (int16 indices, so the 100352-row fp16 table is split into 4 chunks).
  - Scatter-add via one-hot matmuls on the tensor engine: for each 128-edge
    chunk, S[e, d] = (dloc_e == d) * n2_e, aggT_tile += M_chunk^T ... i.e.
    matmul(lhsT=M [e, feat], rhs=S [e, dst]) accumulates PSUM [feat, dst].
  - Position-static structure: per (tile, src-chunk) edge buckets with
    capacities = round128(max bucket size over the 8 cores), so the single
    SPMD instruction stream is valid for every core; padding slots gather
    row 0 with n2 = 0.
  - Self loops are handled as 128 extra "edges" per tile, loaded from the
    core's own x shard with a plain strided DMA (no gather needed).
  - Normalization n2 computed on device from staged integer degree counts:
    n2 = fac / sqrt((cs+2)(cd+2)), fac in {0 (pad), 1 (edge), 2 (self)}.
  - Final per-tile-pair matmul with W in fp32, bias via ACT, output stored
    transposed [128 feat, nodes]; host transposes back.
"""

import sys

sys.path.insert(0, "/opt/trn_rl_repo")

import numpy as np

import concourse.bacc as bacc
import concourse.mybir as mybir
import concourse.tile as tile

F32 = mybir.dt.float32
F16 = mybir.dt.float16
I16 = mybir.dt.int16

N = 100000
FEAT = 128
NCORES = 8
PC = 12544            # nodes per core
NPAD = PC * NCORES    # 100352
TILES = PC // 128     # 98
NCHUNK = 4
CHUNK = NPAD // NCHUNK  # 25088 rows per gather chunk
VT = 8                # tiles per compute wave (PSUM pairs)
GVT = 32              # tiles per gather-call group
ACT_EVERY = 4         # every ACT_EVERY'th S-build goes to the scalar engine


def _round128(x):
    return int(-(-int(x) // 128) * 128)


def build_plan(src, dst, cnt):
    """Host-side integer metadata. src/dst: int64 [E]; cnt: int64 [N] in-degree.

    Bucket capacities are the max bucket size over the 8 cores rounded to 16
    (not 128), so almost no padding rows are gathered. 128-edge matmul chunks
    may straddle two adjacent buckets (= two tiles); the parity of a bucket's
    ordinal within its call is encoded into dloc (+128 for odd) so the two
    one-hot matmuls of a straddling chunk can address their own tile.
    """
    E = src.shape[0]
    core = dst // PC
    dl = dst - core * PC          # 0..PC-1
    t = dl >> 7                   # tile in core
    c = src // CHUNK              # source chunk
    flat = (core * TILES + t) * NCHUNK + c
    bc = np.bincount(flat, minlength=NCORES * TILES * NCHUNK).reshape(
        NCORES, TILES, NCHUNK
    )
    B = bc.max(axis=0)            # [TILES, NCHUNK]
    B = ((B + 15) // 16) * 16     # capacities, multiple of 16 (0 stays 0)

    waves = [list(range(w * VT, min((w + 1) * VT, TILES))) for w in range((TILES + VT - 1) // VT)]
    gwaves = [list(range(g * GVT, min((g + 1) * GVT, TILES))) for g in range((TILES + GVT - 1) // GVT)]

    # --- slot layout (same for every core) ---
    # per wave: [call c=0][call c=1][call c=2][call c=3][self region]
    bucket_base = np.zeros((TILES, NCHUNK), np.int64)  # global slot base
    parity = np.zeros((TILES, NCHUNK), np.int64)
    call_nidx = []   # [wave][chunk] -> num idxs (mult of 128)
    call_slot = []   # [wave][chunk] -> slot base
    call_g16 = []    # [wave][chunk] -> eidx col16 base
    chunk_mms = []   # [wave][chunk] -> list per 128-chunk of [(tile, side)]
    self_slot = []   # [wave] -> slot base of self region
    pos = 0
    g16 = 0
    for wave in gwaves:
        nidx_w, slot_w, g_w, mm_w = [], [], [], []
        for cc in range(NCHUNK):
            nonempty = [tt for tt in wave if B[tt, cc] > 0]
            raw = int(sum(B[tt, cc] for tt in nonempty))
            nidx = _round128(raw)
            slot_w.append(pos)
            g_w.append(g16)
            nidx_w.append(nidx)
            # bucket spans (call-local) and parity
            spans = []
            off = 0
            for o, tt in enumerate(nonempty):
                bucket_base[tt, cc] = pos + off
                parity[tt, cc] = o % 2
                spans.append((off, off + int(bc[:, tt, cc].max()), tt, o % 2))
                off += int(B[tt, cc])
            # per-128-chunk real-bucket intersections
            mms = []
            for j in range(nidx // 128):
                lo, hi = j * 128, (j + 1) * 128
                hit = [(tt, par) for (s0, s1, tt, par) in spans
                       if not (s1 <= lo or s0 >= hi)]
                assert len(hit) <= 2, (len(hit), j, cc)
                mms.append(hit)
            mm_w.append(mms)
            pos += nidx
            g16 += nidx // 16
        call_nidx.append(nidx_w)
        call_slot.append(slot_w)
        call_g16.append(g_w)
        chunk_mms.append(mm_w)
        self_slot.append(pos)
        pos += len(wave) * 128
    total_slots = pos
    gcols16 = g16
    cols = total_slots // 128

    # --- per-core arrays ---
    cnt_pad = np.zeros(NPAD, np.int64)
    cnt_pad[:N] = cnt
    eidx_flat = np.zeros((NCORES, gcols16 * 16), np.int16)
    dloc_flat = np.zeros((NCORES, total_slots), np.float16)
    cs_flat = np.zeros((NCORES, total_slots), np.float16)
    cd_flat = np.zeros((NCORES, total_slots), np.float16)
    fac_flat = np.zeros((NCORES, total_slots), np.float16)

    is_self = np.zeros(total_slots, bool)
    for w, wave in enumerate(gwaves):
        is_self[self_slot[w] : self_slot[w] + len(wave) * 128] = True
    g_of_slot = np.cumsum(~is_self) - 1

    order_all = np.argsort(core * (TILES * NCHUNK) + t * NCHUNK + c, kind="stable")
    flat_sorted = flat[order_all]
    starts = np.searchsorted(flat_sorted, np.arange(NCORES * TILES * NCHUNK), side="left")
    rank = np.arange(E) - starts[flat_sorted]
    bb_flat = np.broadcast_to(bucket_base, (NCORES, TILES, NCHUNK)).reshape(-1)
    par_flat = np.broadcast_to(parity, (NCORES, TILES, NCHUNK)).reshape(-1)
    slots_sorted = bb_flat[flat_sorted] + rank
    par_sorted = par_flat[flat_sorted]
    cores_sorted = core[order_all]
    src_sorted = src[order_all]
    dst_sorted = dst[order_all]
    c_sorted = c[order_all]
    for m in range(NCORES):
        sel = cores_sorted == m
        sl = slots_sorted[sel]
        eidx_flat[m, g_of_slot[sl]] = (src_sorted[sel] - c_sorted[sel] * CHUNK).astype(np.int16)
        dloc_flat[m, sl] = ((dst_sorted[sel] & 127) + 128 * par_sorted[sel]).astype(np.float16)
        cs_flat[m, sl] = cnt_pad[src_sorted[sel]].astype(np.float16)
        cd_flat[m, sl] = cnt_pad[dst_sorted[sel]].astype(np.float16)
        fac_flat[m, sl] = 1.0

    for w, wave in enumerate(gwaves):
        nsw = len(wave) * 128
        sl = self_slot[w] + np.arange(nsw)
        nodes_l = wave[0] * 128 + np.arange(nsw)
        for m in range(NCORES):
            nodes_g = m * PC + nodes_l
            real = nodes_g < N
            dloc_flat[m, sl] = (nodes_l & 127).astype(np.float16)
            cs_flat[m, sl] = cnt_pad[np.minimum(nodes_g, N - 1)].astype(np.float16) * real
            cd_flat[m, sl] = cs_flat[m, sl]
            fac_flat[m, sl] = np.where(real, np.float16(2.0), np.float16(0.0))

    def wrap(a):
        return np.ascontiguousarray(a.reshape(-1, 128).T)

    eidx = np.zeros((NCORES, 128, gcols16), np.int16)
    for m in range(NCORES):
        w16 = eidx_flat[m].reshape(-1, 16).T
        eidx[m] = np.tile(w16, (8, 1))

    return dict(
        B=B, waves=waves, gwaves=gwaves, call_nidx=call_nidx, call_slot=call_slot,
        call_g16=call_g16, self_slot=self_slot, total_slots=total_slots,
        gcols16=gcols16, cols=cols, bucket_base=bucket_base,
        chunk_mms=chunk_mms,
        eidx=eidx,
        dloc=np.stack([wrap(dloc_flat[m]) for m in range(NCORES)]),
        cs=np.stack([wrap(cs_flat[m]) for m in range(NCORES)]),
        cd=np.stack([wrap(cd_flat[m]) for m in range(NCORES)]),
        fac=np.stack([wrap(fac_flat[m]) for m in range(NCORES)]),
    )


def build_bass(plan, repeat=1, mode="full", qspread=False):
    """Build the SPMD Bass program for the static structure in `plan`."""
    B = plan["B"]
    waves = plan["waves"]
    cols = plan["cols"]
    gcols16 = plan["gcols16"]

    nc = bacc.Bacc("TRN2", target_bir_lowering=False, debug=False)
    xt = nc.dram_tensor("xt", [NPAD, FEAT], F16, kind="ExternalInput")
    xself = nc.dram_tensor("xself", [PC, FEAT], F16, kind="ExternalInput")
    eidx_d = nc.dram_tensor("eidx", [128, gcols16], I16, kind="ExternalInput")
    dloc_d = nc.dram_tensor("dloc", [128, cols], F16, kind="ExternalInput")
    cs_d = nc.dram_tensor("cs", [128, cols], F16, kind="ExternalInput")
    cd_d = nc.dram_tensor("cd", [128, cols], F16, kind="ExternalInput")
    fac_d = nc.dram_tensor("fac", [128, cols], F16, kind="ExternalInput")
    w_d = nc.dram_tensor("w", [FEAT, FEAT], F32, kind="ExternalInput")
    bcol_d = nc.dram_tensor("bcol", [FEAT, 1], F32, kind="ExternalInput")
    iota_d = nc.dram_tensor("iota", [128, 256], F16, kind="ExternalInput")
    outT = nc.dram_tensor("outT", [FEAT, PC], F32, kind="ExternalOutput")

    with tile.TileContext(nc) as tc:
        with (
            tc.tile_pool(name="meta", bufs=1) as meta,
            tc.tile_pool(name="mg", bufs=6) as mgp,
            tc.tile_pool(name="ms", bufs=4) as msp,
            tc.tile_pool(name="sp", bufs=16) as spool,
            tc.tile_pool(name="fin", bufs=6) as fin,
            tc.tile_pool(name="aggps", bufs=6, space="PSUM") as aggps,
            tc.tile_pool(name="outps", bufs=2, space="PSUM") as outps,
        ):
            # ---- prologue: metadata loads + bulk normalization ----
            sb_eidx = meta.tile([128, gcols16], I16, tag="eidx")
            nc.sync.dma_start(sb_eidx[:], eidx_d[:])
            sb_dloch = meta.tile([128, cols], F16, tag="dloch")
            nc.sync.dma_start(sb_dloch[:], dloc_d[:])
            sb_cs = meta.tile([128, cols], F16, tag="csh")
            nc.sync.dma_start(sb_cs[:], cs_d[:])
            sb_cd = meta.tile([128, cols], F16, tag="cdh")
            nc.sync.dma_start(sb_cd[:], cd_d[:])
            sb_fac = meta.tile([128, cols], F16, tag="fach")
            nc.sync.dma_start(sb_fac[:], fac_d[:])
            sb_w = meta.tile([FEAT, FEAT], F32, tag="w")
            nc.sync.dma_start(sb_w[:], w_d[:])
            sb_bcol = meta.tile([FEAT, 1], F32, tag="bcol")
            nc.sync.dma_start(sb_bcol[:], bcol_d[:])
            sb_iota = meta.tile([128, 256], F16, tag="iota")
            nc.sync.dma_start(sb_iota[:], iota_d[:])

            sb_dloc = meta.tile([128, cols], F32, tag="dlocf")
            nc.vector.tensor_copy(sb_dloc[:], sb_dloch[:])
            sb_t1 = meta.tile([128, cols], F32, tag="t1")
            nc.vector.tensor_scalar_add(sb_t1[:], sb_cs[:], 2.0)
            sb_t2 = meta.tile([128, cols], F32, tag="t2")
            nc.vector.tensor_scalar_add(sb_t2[:], sb_cd[:], 2.0)
            nc.vector.tensor_mul(sb_t1[:], sb_t1[:], sb_t2[:])
            nc.scalar.activation(sb_t2[:], sb_t1[:], mybir.ActivationFunctionType.Sqrt)
            nc.vector.reciprocal(sb_t1[:], sb_t2[:])
            sb_facf = meta.tile([128, cols], F32, tag="facf")
            nc.vector.tensor_copy(sb_facf[:], sb_fac[:])
            sb_n2 = meta.tile([128, cols], F32, tag="n2")
            nc.vector.tensor_mul(sb_n2[:], sb_t1[:], sb_facf[:])
            sb_n2n = meta.tile([128, cols], F32, tag="n2n")
            nc.vector.tensor_scalar_mul(sb_n2n[:], sb_n2[:], -1.0)

            sb_count = [0]
            import contextlib
            loop_cm = tc.For_i(0, repeat, 1) if repeat > 1 else contextlib.nullcontext()

            def build_s(gcol, side=0):
                """S[e, d] = (dloc[e] == side*128 + d) * n2[e] for the
                128-edge chunk at global column gcol. Alternates DVE / ACT."""
                sb_count[0] += 1
                io = sb_iota[:, side * 128 : side * 128 + 128]
                if sb_count[0] % ACT_EVERY == 0:
                    ta = spool.tile([128, 128], F16, tag="sa")
                    nc.scalar.activation(
                        ta[:], io, mybir.ActivationFunctionType.Abs,
                        bias=sb_dloc[:, gcol : gcol + 1], scale=-1.0,
                    )
                    s = spool.tile([128, 128], F16, tag="sb")
                    nc.scalar.activation(
                        s[:], ta[:], mybir.ActivationFunctionType.Relu,
                        bias=sb_n2[:, gcol : gcol + 1],
                        scale=sb_n2n[:, gcol : gcol + 1],
                    )
                else:
                    s = spool.tile([128, 128], F16, tag="sb")
                    nc.vector.tensor_scalar(
                        s[:], io,
                        sb_dloc[:, gcol : gcol + 1], sb_n2[:, gcol : gcol + 1],
                        mybir.AluOpType.is_equal, mybir.AluOpType.mult,
                    )
                return s

            # ---- main loop: gather groups (gwaves) / compute waves ----
            with loop_cm:
              for g, gtiles in enumerate(plan["gwaves"]):
                  mtiles = {}
                  for cc in range(NCHUNK):
                      nidx = plan["call_nidx"][g][cc]
                      if nidx == 0:
                          continue
                      m = mgp.tile([128, nidx // 128, 128], F16, tag="mg")
                      g16 = plan["call_g16"][g][cc]
                      nc.gpsimd.dma_gather(
                          m[:, : nidx // 128, :],
                          xt[cc * CHUNK : (cc + 1) * CHUNK, :],
                          sb_eidx[:, g16 : g16 + nidx // 16],
                          nidx, nidx, FEAT,
                          single_packet=(nidx <= 1024),
                      )
                      mtiles[cc] = m
                  nsw = len(gtiles)
                  ms = msp.tile([128, nsw, 128], F16, tag="ms")
                  r0 = gtiles[0] * 128
                  nc.sync.dma_start(
                      ms[:], xself[r0 : r0 + nsw * 128, :].rearrange("(n p) d -> p n d", p=128)
                  )

                  if mode == "gather":
                      o = fin.tile([128, 128], F32, tag="gonly", name="gonly")
                      nc.vector.tensor_copy(o[:], ms[:, 0, :])
                      nc.sync.dma_start(outT[:, gtiles[0] * 128 : gtiles[0] * 128 + 128], o[:])
                      for cc in range(NCHUNK):
                          if cc in mtiles:
                              o2 = fin.tile([128, 128], F32, tag="gonly2", name="gonly2")
                              nc.vector.tensor_copy(o2[:], mtiles[cc][:, 0, :])
                              nc.sync.dma_start(outT[:, gtiles[0] * 128 : gtiles[0] * 128 + 128], o2[:])
                      continue

                  per_tile = {t: [] for t in gtiles}
                  for cc in range(NCHUNK):
                      if plan["call_nidx"][g][cc] == 0:
                          continue
                      call_base = plan["call_slot"][g][cc]
                      for j, hits in enumerate(plan["chunk_mms"][g][cc]):
                          gcol = (call_base + j * 128) // 128
                          for (t, side) in hits:
                              per_tile[t].append((cc, j, gcol, side))

                  for cw0 in range(0, len(gtiles), VT):
                      cwave = gtiles[cw0 : cw0 + VT]
                      pairs = {}
                      started = {}

                      def pair_half(t, cwave=cwave, pairs=pairs):
                          ti = t - cwave[0]
                          pi = ti // 2
                          if pi not in pairs:
                              pairs[pi] = aggps.tile([128, 256], F32, tag="agg", name="agg")
                          return pairs[pi], (ti % 2) * 128

                      for t in cwave:
                          for (cc, j, gcol, side) in per_tile[t]:
                              ppair, half = pair_half(t)
                              s = build_s(gcol, side)
                              nc.tensor.matmul(
                                  ppair[:, half : half + 128],
                                  mtiles[cc][:, j, :],
                                  s[:],
                                  start=(t not in started), stop=False,
                                  skip_group_check=True,
                              )
                              started[t] = True
                          ppair, half = pair_half(t)
                          tig = t - gtiles[0]
                          gcol = (plan["self_slot"][g] + tig * 128) // 128
                          s = build_s(gcol, 0)
                          nc.tensor.matmul(
                              ppair[:, half : half + 128], ms[:, tig, :], s[:],
                              start=(t not in started), stop=True, skip_group_check=True,
                          )

                      for pi in sorted(pairs):
                          ppair = pairs[pi]
                          asb = fin.tile([128, 256], F32, tag="asb")
                          nc.vector.tensor_copy(asb[:], ppair[:])
                          op = outps.tile([128, 256], F32, tag="op")
                          nc.tensor.matmul(op[:], sb_w[:], asb[:], skip_group_check=True)
                          osb = fin.tile([128, 256], F32, tag="osb")
                          nc.scalar.activation(
                              osb[:], op[:], mybir.ActivationFunctionType.Identity,
                              bias=sb_bcol[:],
                          )
                          base = (cwave[0] + pi * 2) * 128
                          nc.sync.dma_start(outT[:, base : base + 256], osb[:])
    nc.compile()
    return nc


_CACHE = {}


def _get_compiled(src, dst, cnt):
    plan = build_plan(src, dst, cnt)
    key = (plan["total_slots"], plan["B"].tobytes())
    if key not in _CACHE:
        _CACHE[key] = (build_bass(plan), plan)
    else:
        _CACHE[key] = (_CACHE[key][0], plan)
    return _CACHE[key]


def kernel(x, edge_index, W, b):
    from concourse.bass_utils import run_bass_kernel_spmd

    x = np.asarray(x)
    edge_index = np.asarray(edge_index)
    W = np.asarray(W)
    b = np.asarray(b)
    src = edge_index[0].astype(np.int64)
    dst = edge_index[1].astype(np.int64)
    cnt = np.bincount(dst, minlength=N)

    nc, plan = _get_compiled(src, dst, cnt)

    xt = np.zeros((NPAD, FEAT), np.float16)
    xt[:N] = x.astype(np.float16)
    iota = np.tile(np.arange(256, dtype=np.float16), (128, 1))
    wf = W.astype(np.float32)
    bcol = b.astype(np.float32).reshape(FEAT, 1)

    in_maps = []
    for m in range(NCORES):
        in_maps.append({
            "xt": xt,
            "xself": xt[m * PC : (m + 1) * PC],
            "eidx": plan["eidx"][m],
            "dloc": plan["dloc"][m],
            "cs": plan["cs"][m],
            "cd": plan["cd"][m],
            "fac": plan["fac"][m],
            "w": wf,
            "bcol": bcol,
            "iota": iota,
        })
    res = run_bass_kernel_spmd(nc, in_maps, list(range(NCORES)))
    outT = np.concatenate([res.results[m]["outT"] for m in range(NCORES)], axis=1)
    return np.ascontiguousarray(outT[:, :N].T).astype(np.float32)



# revision 5
# speedup vs baseline: 2.1415x; 2.1415x over previous
"""GCN layer (improved self-loops) on 8 Trainium2 NeuronCores — v2.

out = D^{-1/2} (A + 2I) D^{-1/2} X W + b,  deg = in_count + 2.

Key HW facts driving this design (measured on TRN2):
  - SWDGE dma_gather descriptor generation on the GpSimd Q7 costs ~7.7ns
    per *index* regardless of element size -> gather 512B pairs (2 node
    rows per descriptor) instead of 256B singles to halve descgen time.
  - SWDGE descgen holds the shared SBUF port pair for the whole
    instruction, fully blocking concurrent DVE perf-mode ops -> do NOT
    build one-hot scatter matrices on the vector engine; stage them
    pre-built from the host and stream via HWDGE DMA (SDMA engines are
    ~15% busy, bandwidth is free).
  - PE cost per 128-col matmul is ~456ns (LDW+MM) -> align buckets to
    128 pair-slots so every 128-slot chunk maps to exactly one dst tile
    (no straddle duplication).

Structure (SPMD, one program for all 8 cores; only input data differs):
  - Nodes sharded by destination: core m owns rows [m*12544, (m+1)*12544).
  - Edges bucketed per (dst tile, src chunk); bucket edges are paired;
    each pair becomes one 512B gather descriptor reading two adjacent
    rows of a per-core Euler-walk ordered table (duplication ~6%).
  - Aggregation: for each 128-pair chunk q (owned by one dst tile):
      psum[f, d] += M2[:, q, 0:128]^T @ S_A   (A-side edges)
      psum[f, d] += M2[:, q, 128:256]^T @ S_B (B-side edges)
    where S_A/S_B are host-staged [128, 128] fp16 matrices carrying the
    full gcn_norm weight n2 = dinv[src]*dinv[dst] at [pair_slot, dst&127].
  - Self loops: per-tile staged diag(2*dinv^2) matrices against a plain
    strided load of the core's own x rows.
  - Then out = W^T agg per psum pair; bias is added on the host.
"""

import sys

sys.path.insert(0, "/opt/trn_rl_repo")

import numpy as np

import bass_rust
import concourse.bacc as bacc
import concourse.mybir as mybir
import concourse.tile as tile

F16 = mybir.dt.float16
F32 = mybir.dt.float32
I16 = mybir.dt.int16

N = 100000
FEAT = 128
NCORES = 8
PC = 12544            # nodes per core
NPAD = PC * NCORES    # 100352
TILES = PC // 128     # 98
NCHUNK = 4
CHUNK = NPAD // NCHUNK  # 25088 source rows per chunk sub-table
GVT = 14              # tiles per gather/compute wave (98 = 7*14)


def _build_walks(pairs_full, singles):
    """Euler-trail decomposition of the pair multigraph of one (core, chunk).

    pairs_full: list of (sA, sB, pid) — full pairs (local src ids).
    singles:    list of (sA, pid) — odd-bucket leftovers (B side is padding).
    Returns (rows, idx_of_pid) where rows is the sub-table row order and
    idx_of_pid[pid] = (table position, flipped) — flipped means the walk
    traversed the pair sB->sA so the A-side row is sB.

    Method: per connected component, pair up odd-degree vertices with
    virtual edges, build an Euler circuit (Hierholzer, valid since all
    degrees even), then split the circuit at the virtual edges.
    """
    nreal = len(pairs_full)
    adj = {}  # vertex -> list of (edge_id, other)
    edges = []  # (a, b) incl. virtual

    def add_edge(a, b):
        eid = len(edges)
        edges.append((a, b))
        adj.setdefault(a, []).append((eid, b))
        adj.setdefault(b, []).append((eid, a))
        return eid

    for (a, b, _pid) in pairs_full:
        add_edge(a, b)

    # connected components + odd vertices per component
    comp = {}
    comps = []
    for v0 in adj:
        if v0 in comp:
            continue
        cid = len(comps)
        stack = [v0]
        comp[v0] = cid
        verts = [v0]
        while stack:
            v = stack.pop()
            for (_e, w) in adj[v]:
                if w not in comp:
                    comp[w] = cid
                    verts.append(w)
                    stack.append(w)
        comps.append(verts)

    for verts in comps:
        odd = [v for v in verts if len(adj[v]) & 1]
        for i in range(0, len(odd), 2):
            add_edge(odd[i], odd[i + 1])

    used = np.zeros(len(edges), dtype=bool)
    ptr = {v: 0 for v in adj}
    rows = []
    idx_of_pid = {}

    for verts in comps:
        v0 = verts[0]
        # Hierholzer Euler circuit from v0 (all degrees even now)
        stack = [(v0, -1)]
        trail_v = []
        trail_e = []
        while stack:
            v, _ = stack[-1]
            al = adj[v]
            p = ptr[v]
            while p < len(al) and used[al[p][0]]:
                p += 1
            ptr[v] = p
            if p < len(al):
                eid, w = al[p]
                used[eid] = True
                stack.append((w, eid))
            else:
                vv, ee = stack.pop()
                trail_v.append(vv)
                if ee >= 0:
                    trail_e.append(ee)
        trail_v.reverse()
        trail_e.reverse()
        L = len(trail_e)
        if L == 0:
            continue
        # rotate so a virtual edge (if any) is last
        virt_pos = [i for i, e in enumerate(trail_e) if e >= nreal]
        if virt_pos:
            p = virt_pos[0]
            trail_e = trail_e[p + 1 :] + trail_e[: p + 1]
            trail_v = trail_v[p + 1 : L + 1] + trail_v[1 : p + 2]
        # emit segments split at virtual edges
        seg_start = 0  # index into trail_e
        i = 0
        while i <= L:
            if i == L or trail_e[i] >= nreal:
                if i > seg_start:
                    base = len(rows)
                    rows.extend(trail_v[seg_start : i + 1])
                    for k in range(seg_start, i):
                        eid = trail_e[k]
                        a, b, pid = pairs_full[eid]
                        flipped = trail_v[k] != a
                        idx_of_pid[pid] = (base + (k - seg_start), flipped)
                seg_start = i + 1
            i += 1

    # singles: need any table position whose row == sA
    pos_of = {}
    for i, r in enumerate(rows):
        if r not in pos_of:
            pos_of[r] = i
    for (a, pid) in singles:
        if a in pos_of:
            idx_of_pid[pid] = (pos_of[a], False)
        else:
            pos_of[a] = len(rows)
            idx_of_pid[pid] = (len(rows), False)
            rows.append(a)
    return rows, idx_of_pid


def build_plan(src, dst, cnt):
    """Host-side metadata. src/dst int64 [E]; cnt int64 [N] in-degree."""
    E = src.shape[0]
    core = dst // PC
    tl = (dst % PC) >> 7          # dst tile within core
    ch = src // CHUNK             # source chunk

    dinv = np.zeros(NPAD, np.float64)
    dinv[:N] = 1.0 / np.sqrt(cnt + 2.0)

    flat = (core * TILES + tl) * NCHUNK + ch
    bc = np.bincount(flat, minlength=NCORES * TILES * NCHUNK).reshape(
        NCORES, TILES, NCHUNK)
    pairs_c = -(-bc // 2)                        # ceil(bucket/2) per core
    cap = pairs_c.max(axis=0)                    # [TILES, NCHUNK] max pairs
    cap = np.maximum(-(-cap // 128) * 128, 128)  # pair-slot capacity, mult 128

    waves = [list(range(w * GVT, min((w + 1) * GVT, TILES)))
             for w in range(-(-TILES // GVT))]

    # ---- static chunk layout (shared across cores) ----
    # order: wave g -> chunk c -> tile t (in wave) -> bucket 128-block
    # chunk ordinal q; pair-slot s in [128q, 128q+128)
    bucket_base = np.zeros((TILES, NCHUNK), np.int64)  # first pair-slot
    call_nidx = []    # [g][c] -> num pair-slots (mult of 128)
    call_g16 = []     # [g][c] -> eidx col16 base
    tile_chunks = [[] for _ in range(TILES)]  # t -> [(c, q, j_local)]
    q = 0
    g16 = 0
    pos = 0
    for g, wave in enumerate(waves):
        nidx_w, g16_w = [], []
        for c in range(NCHUNK):
            nidx = int(sum(cap[t, c] for t in wave))
            nidx_w.append(nidx)
            g16_w.append(g16)
            j = 0
            for t in wave:
                bucket_base[t, c] = pos
                for _blk in range(cap[t, c] // 128):
                    tile_chunks[t].append((c, q, j))
                    q += 1
                    j += 1
                    pos += 128
            g16 += nidx // 16
        call_nidx.append(nidx_w)
        call_g16.append(g16_w)
    total_pairs = pos
    nq = q
    gcols16 = g16

    # ---- per-core data ----
    # assign each edge to (pair slot, side)
    order = np.argsort(flat * np.int64(1), kind="stable")
    flat_s = flat[order]
    starts = np.searchsorted(flat_s, np.arange(NCORES * TILES * NCHUNK))
    rank = np.arange(E) - starts[flat_s]
    bb = np.broadcast_to(bucket_base, (NCORES, TILES, NCHUNK)).reshape(-1)
    slot_s = bb[flat_s] + (rank >> 1)
    side_s = rank & 1
    src_s = src[order]
    dst_s = dst[order]
    core_s = core[order]
    ch_s = ch[order]

    eidx = np.zeros((NCORES, 128, gcols16), np.int16)
    s_host = np.zeros((NCORES, 128, nq * 256), np.float16)
    subcaps = []

    for m in range(NCORES):
        sel = core_s == m
        m_slot = slot_s[sel]
        m_side = side_s[sel]
        m_src = src_s[sel]
        m_dst = dst_s[sel]
        m_ch = ch_s[sel]
        idx_val = np.zeros(total_pairs, np.int64)  # per pair slot
        # A/B edge arrays per slot
        a_src = np.full(total_pairs, -1, np.int64)
        b_src = np.full(total_pairs, -1, np.int64)
        a_dst = np.zeros(total_pairs, np.int64)
        b_dst = np.zeros(total_pairs, np.int64)
        a_src[m_slot[m_side == 0]] = m_src[m_side == 0]
        a_dst[m_slot[m_side == 0]] = m_dst[m_side == 0]
        b_src[m_slot[m_side == 1]] = m_src[m_side == 1]
        b_dst[m_slot[m_side == 1]] = m_dst[m_side == 1]

        core_tabs = []
        for c in range(NCHUNK):
            # slots of this chunk across all waves
            csl = np.concatenate([
                np.arange(bucket_base[t, c], bucket_base[t, c] + cap[t, c])
                for t in range(TILES)])
            full = csl[(a_src[csl] >= 0) & (b_src[csl] >= 0)]
            single = csl[(a_src[csl] >= 0) & (b_src[csl] < 0)]
            pairs_full = [(int(a_src[s] - c * CHUNK), int(b_src[s] - c * CHUNK), int(s))
                          for s in full]
            singles = [(int(a_src[s] - c * CHUNK), int(s)) for s in single]
            rows, idx_of = _build_walks(pairs_full, singles)
            assert len(rows) + 1 < 32768, len(rows)
            core_tabs.append(rows)
            for s in full:
                p, fl = idx_of[int(s)]
                idx_val[s] = p
                if fl:
                    a_src[s], b_src[s] = b_src[s], a_src[s]
                    a_dst[s], b_dst[s] = b_dst[s], a_dst[s]
            for s in single:
                p, _ = idx_of[int(s)]
                idx_val[s] = p
        subcaps.append(core_tabs)

        # eidx wrap-16, replicate 8x across partitions
        w16 = idx_val.reshape(-1, 16).T.astype(np.int16)
        eidx[m] = np.tile(w16, (8, 1))

        # S matrices: [128 part=slot&127, col = q*256 + side*128 + (dst&127)]
        sl = np.arange(total_pairs)
        qq = sl >> 7
        pr = sl & 127
        av = a_src >= 0
        n2a = dinv[np.minimum(a_src, NPAD - 1)] * dinv[a_dst] * av
        bv = b_src >= 0
        n2b = dinv[np.minimum(b_src, NPAD - 1)] * dinv[b_dst] * bv
        s_host[m][pr[av], qq[av] * 256 + (a_dst[av] & 127)] = n2a[av].astype(np.float16)
        s_host[m][pr[bv], qq[bv] * 256 + 128 + (b_dst[bv] & 127)] = n2b[bv].astype(np.float16)

    SUBCAP = max(len(t) + 1 for m in range(NCORES) for t in subcaps[m])
    SUBCAP = int(-(-SUBCAP // 16) * 16)
    assert SUBCAP <= 32767

    # self-loop diag matrices [128, TILES*128]
    s_self = np.zeros((NCORES, 128, TILES * 128), np.float16)
    jj = np.arange(PC)
    for m in range(NCORES):
        g = m * PC + jj
        val = (2.0 * dinv[g] * dinv[g]).astype(np.float16)
        s_self[m][jj & 127, (jj >> 7) * 128 + (jj & 127)] = val

    return dict(
        cap=cap, waves=waves, call_nidx=call_nidx, call_g16=call_g16,
        tile_chunks=tile_chunks, total_pairs=total_pairs, nq=nq,
        gcols16=gcols16, SUBCAP=SUBCAP, subcaps=subcaps,
        eidx=eidx, s_host=s_host, s_self=s_self,
        wave_q0=[min(tile_chunks[t][0][1] for t in wave) for wave in waves],
    )


def build_bass(plan):
    waves = plan["waves"]
    call_nidx = plan["call_nidx"]
    gcols16 = plan["gcols16"]
    SUBCAP = plan["SUBCAP"]
    nq = plan["nq"]

    nc = bacc.Bacc("TRN2", target_bir_lowering=False, debug=False)
    xt = nc.dram_tensor("xt", [NCHUNK * SUBCAP + 1, FEAT], F16, kind="ExternalInput")
    xself = nc.dram_tensor("xself", [PC, FEAT], F16, kind="ExternalInput")
    eidx_d = nc.dram_tensor("eidx", [128, gcols16], I16, kind="ExternalInput")
    s_d = nc.dram_tensor("smat", [128, nq * 256], F16, kind="ExternalInput")
    sself_d = nc.dram_tensor("sself", [128, TILES * 128], F16, kind="ExternalInput")
    w_d = nc.dram_tensor("w", [FEAT, FEAT], F16, kind="ExternalInput")
    outT = nc.dram_tensor("outT", [FEAT, PC], F16, kind="ExternalOutput")

    with tile.TileContext(nc) as tc:
        with (
            tc.tile_pool(name="meta", bufs=1) as meta,
            tc.tile_pool(name="mg", bufs=2) as mgp,
            tc.tile_pool(name="sw", bufs=2) as swp,
            tc.tile_pool(name="ms", bufs=2) as msp,
            tc.tile_pool(name="fin", bufs=4) as fin,
            tc.tile_pool(name="aggps", bufs=6, space="PSUM") as aggps,
            tc.tile_pool(name="outps", bufs=2, space="PSUM") as outps,
        ):
            sb_eidx = meta.tile([128, gcols16], I16, tag="eidx")
            nc.sync.dma_start(sb_eidx[:], eidx_d[:])
            sb_w = meta.tile([FEAT, FEAT], F16, tag="w")
            nc.sync.dma_start(sb_w[:], w_d[:])

            for g, wave in enumerate(waves):
                nsw = len(wave)
                # gathers (SWDGE): 4 calls, 512B pair descriptors
                mtiles = {}
                for c in range(NCHUNK):
                    nidx = call_nidx[g][c]
                    if nidx == 0:
                        continue
                    m2 = mgp.tile([128, nidx // 128, 256], F16, tag=f"mg{c}")
                    g16 = plan["call_g16"][g][c]
                    in_ap = xt[:, :].copy()
                    in_ap.ap = bass_rust.VecI64Pair([(FEAT, SUBCAP), (1, 256)])
                    in_ap.offset = c * SUBCAP * FEAT
                    nc.gpsimd.dma_gather(
                        m2[:, : nidx // 128, :],
                        in_ap,
                        sb_eidx[:, g16 : g16 + nidx // 16],
                        nidx, nidx, 256,
                        elem_step=FEAT,
                        single_packet=(nidx <= 1024),
                    )
                    mtiles[c] = m2

                # HWDGE loads: S block, self rows, self S
                q0 = plan["wave_q0"][g]
                nqw = sum(call_nidx[g]) // 128
                sw = swp.tile([128, nqw * 256], F16, tag="sw")
                nc.sync.dma_start(sw[:], s_d[:, q0 * 256 : (q0 + nqw) * 256])
                ms = msp.tile([128, nsw, 128], F16, tag="ms")
                r0 = wave[0] * 128
                nc.sync.dma_start(
                    ms[:], xself[r0 : r0 + nsw * 128, :].rearrange(
                        "(n p) d -> p n d", p=128))
                ssl = msp.tile([128, nsw * 128], F16, tag="ssl")
                nc.sync.dma_start(
                    ssl[:], sself_d[:, wave[0] * 128 : (wave[0] + nsw) * 128])

                for p0 in range(0, nsw, 2):
                    t0, t1 = wave[p0], wave[p0 + 1]
                    ppair = aggps.tile([128, 256], F32, tag="agg", name="agg")
                    for half, t in ((0, t0), (128, t1)):
                        first = True
                        for (c, qq, j) in plan["tile_chunks"][t]:
                            lq = qq - q0
                            for side in (0, 1):
                                nc.tensor.matmul(
                                    ppair[:, half : half + 128],
                                    mtiles[c][:, j, side * 128 : side * 128 + 128],
                                    sw[:, lq * 256 + side * 128 : lq * 256 + side * 128 + 128],
                                    start=first, stop=False,
                                    skip_group_check=True,
                                )
                                first = False
                        tig = t - wave[0]
                        nc.tensor.matmul(
                            ppair[:, half : half + 128],
                            ms[:, tig, :],
                            ssl[:, tig * 128 : tig * 128 + 128],
                            start=first, stop=True, skip_group_check=True,
                        )
                    asb = fin.tile([128, 256], F16, tag="asb")
                    nc.scalar.activation(
                        asb[:], ppair[:], mybir.ActivationFunctionType.Identity)
                    op = outps.tile([128, 256], F32, tag="op")
                    nc.tensor.matmul(op[:], sb_w[:], asb[:], skip_group_check=True)
                    osb = fin.tile([128, 256], F16, tag="osb")
                    nc.scalar.activation(
                        osb[:], op[:], mybir.ActivationFunctionType.Identity)
                    nc.sync.dma_start(outT[:, t0 * 128 : t0 * 128 + 256], osb[:])
    nc.compile()
    return nc


_CACHE = {}


def _get_compiled(src, dst, cnt):
    plan = build_plan(src, dst, cnt)
    key = (plan["SUBCAP"], plan["cap"].tobytes())
    if key not in _CACHE:
        _CACHE[key] = (build_bass(plan), plan)
    else:
        _CACHE[key] = (_CACHE[key][0], plan)
    return _CACHE[key]


def make_inputs(plan, x, W):
    """Per-core input maps (everything except the run itself)."""
    xf = np.asarray(x).astype(np.float16)
    SUBCAP = plan["SUBCAP"]
    in_maps = []
    for m in range(NCORES):
        xtab = np.zeros((NCHUNK * SUBCAP + 1, FEAT), np.float16)
        for c in range(NCHUNK):
            rows = np.asarray(plan["subcaps"][m][c], np.int64)
            if rows.size:
                xtab[c * SUBCAP : c * SUBCAP + rows.size] = xf[
                    np.minimum(rows + c * CHUNK, N - 1)] * (rows + c * CHUNK < N)[:, None].astype(np.float16)
        xs = np.zeros((PC, FEAT), np.float16)
        lo = m * PC
        hi = min((m + 1) * PC, N)
        xs[: hi - lo] = xf[lo:hi]
        in_maps.append({
            "xt": xtab,
            "xself": xs,
            "eidx": plan["eidx"][m],
            "smat": plan["s_host"][m],
            "sself": plan["s_self"][m],
            "w": np.asarray(W).astype(np.float16),
        })
    return in_maps


def kernel(x, edge_index, W, b):
    from concourse.bass_utils import run_bass_kernel_spmd

    x = np.asarray(x)
    edge_index = np.asarray(edge_index)
    W = np.asarray(W)
    b = np.asarray(b)
    src = edge_index[0].astype(np.int64)
    dst = edge_index[1].astype(np.int64)
    cnt = np.bincount(dst, minlength=N)

    nc, plan = _get_compiled(src, dst, cnt)
    in_maps = make_inputs(plan, x, W)
    res = run_bass_kernel_spmd(nc, in_maps, list(range(NCORES)))
    outT = np.concatenate([res.results[m]["outT"] for m in range(NCORES)], axis=1)
    return (outT[:, :N].T.astype(np.float32) + b.astype(np.float32)[None, :])
